# revision 24
# baseline (speedup 1.0000x reference)
"""CSwin vertical-stripe window attention (sparse_attention) on 8 TRN2 cores.

Sharding: data-parallel over batch B=8 (one image per NeuronCore). No
collectives. Per-core kernel computes windowed attention + output
projection for one [4096, 256] image; the tiny LePE depthwise 3x3 conv
(0.7% of FLOPs) is folded host-side into a per-window additive plane.

v7 design (softmax-denominator matmuls eliminated via augmented V):
 - The PV stationary for each head pair is widened to 128 cols:
   [ones(32) | ones(32) | v_even(32) | v_odd(32)] with disjoint 32-col
   blocks per slot, so ONE set of PV matmuls yields both the attention
   output (PSUM rows 64-128) and 32 duplicated copies of each softmax
   denominator (rows 0-64) -- the v6 SM matmuls (1/3 of all PE columns)
   are gone at zero extra PE cost (PE time ~ moving columns only).
 - finish: one reciprocal_approx_fast [64,512] on the den rows (base-0,
   custom-DVE ops require base-0 partition addressing) + one cross-base
   tensor_tensor multiply + one lepe add per head pair.
 - QK^T bf16, 4-head row-packed, split across TWO 2-bank PSUM tiles
   (heads 01 -> bigA, heads 23 -> bigB); fine-grained software pipeline
   emits, per jc-slot, this pair's QK+exp then the previous pair's
   PV-hat quarter and a proj piece, so no engine queue head blocks.
 - exp: Scalar-engine ACTIVATE for most tiles; some halves use a
   single-instruction DVE Schraudolph (fp16-bit trick: round(x*a+b) as
   int16 IS the fp16 exp, ~3% max rel err, bias cancels in softmax).
 - Window 7 (shifted stripes) is block-diagonal: masked quadrants are
   never computed (N=256 matmuls, strided Exp).
 - proj bias added by the DVE PSUM-evacuation op; output bf16
   window-major contiguous, host un-permutes.
"""
import numpy as np
import ml_dtypes

import concourse.bass as bass
import concourse.bacc as bacc
import concourse.mybir as mybir
import concourse.tile as tile

RESO, STRIPE, DIM, NH, HD = 64, 8, 256, 8, 32
B, L, WIN, NW = 8, RESO * RESO, RESO * STRIPE, RESO // STRIPE
P = 128
F32, BF16 = mybir.dt.float32, mybir.dt.bfloat16
F16, I16 = mybir.dt.float16, mybir.dt.int16

# fused per-window input blob offsets (16-bit elements, per partition)
O_QT, O_KT, O_VN, O_LP = 0, 1024, 2048, 6144
WCOLS = 7168

Exp = mybir.ActivationFunctionType.Exp
# Schraudolph fp16 exp: fp16_bits(e^x) ~ round(x * SCH_A + SCH_B)
SCH_A, SCH_B = 1477.3197218702985, 15315.5


def _dve_exp_half(w, g, jc, h0):
    """Which exp halves run on the Vector engine instead of Scalar.

    B-halves only: the offload's pipeline win is freeing bigB early so
    the next QK group unblocks; A-half offloads only added Vector-engine
    contention."""
    return w != NW - 1 and jc == 1 and h0 == 2


def build_nc():
    nc = bacc.Bacc("TRN2", target_bir_lowering=False, debug=False)
    win = nc.declare_dram_parameter("win", [NW, P, WCOLS], BF16, isOutput=False)
    pw = nc.declare_dram_parameter("pw", [P, 2 * DIM], BF16, isOutput=False)
    pb = nc.declare_dram_parameter("pb", [P, 2], F32, isOutput=False)
    # transposed output: [w, c-chunk, c, q]; host un-permutes
    out = nc.declare_dram_parameter("out", [NW, 2, P, 512], BF16,
                                    isOutput=True)

    with tile.TileContext(nc) as tc:
        with tc.tile_pool(name="const", bufs=1) as cp, \
             tc.tile_pool(name="sb", bufs=1) as sp, \
             tc.tile_pool(name="ps", bufs=1, space="PSUM") as pp:
            # ---- first window's qk plane goes out before anything else ----
            wts = {}
            # warmup: ramp PE pstate + pull the ACT table while DMAs run
            wu = cp.tile([P, 128], BF16, name="wu")
            nc.vector.memset(wu[:], 0.0)
            wups = pp.tile([P, 512], F32, name="wups", tag="pj", bufs=1)
            nc.tensor.matmul(wups[:, 0:128], wu[:], wu[:],
                             start=True, stop=True)
            wue = cp.tile([P, 128], F16, name="wue")
            nc.scalar.activation(wue[:], wups[:, 0:128], Exp,
                                 bias=0.0, scale=1.0)

            wt0 = sp.tile([P, WCOLS], BF16, name="wt0", tag="wt", bufs=3)
            nc.sync.dma_start(wt0[:, :512], win[:][0][:, :512])
            nc.sync.dma_start(wt0[:, 1024:1536], win[:][0][:, 1024:1536])
            nc.sync.dma_start(wt0[:, 512:1024], win[:][0][:, 512:1024])
            nc.sync.dma_start(wt0[:, 1536:2048], win[:][0][:, 1536:2048])
            nc.sync.dma_start(wt0[:, 2048:4096], win[:][0][:, 2048:4096])
            nc.sync.dma_start(wt0[:, 4096:], win[:][0][:, 4096:])
            wts[0] = wt0

            # ---- constants ----
            pw_sb = cp.tile([P, 2, 2, P], BF16, name="pw_sb")
            nc.sync.dma_start(pw_sb[:], pw[:].rearrange(
                "p (g k c) -> p g k c", g=2, k=2))
            pb_sb = cp.tile([P, 2], F32, name="pb_sb")
            nc.sync.dma_start(pb_sb[:], pb[:])

            def views(wt):
                return (
                    wt[:, O_QT:O_QT + 1024].rearrange("p (g q) -> p g q", g=2),
                    wt[:, O_KT:O_KT + 1024].rearrange("p (g q) -> p g q", g=2),
                    wt[:, O_VN:O_VN + 4096].bitcast(F16).rearrange(
                        "p (g j h c) -> p g j h c", g=2, j=4, h=4),
                    wt[:, O_LP:O_LP + 1024].rearrange("p (g q) -> p g q", g=2),
                )

            def emit_bg_exp(w, g, jc):
                """One QK jc-quarter (4 row-packed bf16 MMs into the A/B
                PSUM pair) followed by its two exps (Scalar or Vector)."""
                qT, kT, _, _ = views(wts[w])
                bigA = pp.tile([P, 1024], F32, name=f"bA{w}{g}{jc}",
                               tag="bigA", bufs=1)
                bigB = pp.tile([P, 1024], F32, name=f"bB{w}{g}{jc}",
                               tag="bigB", bufs=1)
                eT = sp.tile([P, 2048], F16, name=f"eT{w}{g}{jc}",
                             tag="eT", bufs=12)
                halves = ((bigA, 0), (bigB, 2))
                if w < NW - 1:
                    for big, h0 in halves:
                        for hx in range(2):
                            hp = h0 + hx
                            nc.tensor.matmul(
                                big[:, 512 * hx:512 * (hx + 1)],
                                kT[32 * hp:32 * hp + 32, g,
                                   P * jc:P * (jc + 1)],
                                qT[32 * hp:32 * hp + 32, g, :],
                                start=True, stop=True,
                                tile_position=(32 * hp, 0))
                    for big, h0 in halves:
                        ev = eT[:, 1024 * (h0 // 2):1024 * (h0 // 2) + 1024]
                        if _dve_exp_half(w, g, jc, h0):
                            nc.vector.tensor_scalar(
                                out=ev.bitcast(I16), in0=big[:],
                                scalar1=SCH_A, scalar2=SCH_B,
                                op0=mybir.AluOpType.mult,
                                op1=mybir.AluOpType.add)
                        else:
                            nc.scalar.activation(ev, big[:], Exp,
                                                 bias=0.0, scale=1.0)
                else:
                    # shifted window: block-diagonal mask. keys of
                    # quarter jc only see queries qo..qo+256.
                    qo = 0 if jc < 2 else 256
                    for big, h0 in halves:
                        for hx in range(2):
                            hp = h0 + hx
                            nc.tensor.matmul(
                                big[:, 512 * hx + qo:512 * hx + qo + 256],
                                kT[32 * hp:32 * hp + 32, g,
                                   P * jc:P * (jc + 1)],
                                qT[32 * hp:32 * hp + 32, g, qo:qo + 256],
                                start=True, stop=True,
                                tile_position=(32 * hp, 0))
                    for big, h0 in halves:
                        bv = big[:].rearrange(
                            "p (h q) -> p h q", h=2)[:, :, qo:qo + 256]
                        ev = eT[:, 1024 * (h0 // 2):
                                1024 * (h0 // 2) + 1024].rearrange(
                            "p (h q) -> p h q", h=2)[:, :, qo:qo + 256]
                        nc.scalar.activation(ev, bv, Exp,
                                             bias=0.0, scale=1.0)
                return eT

            pvh_of = {}

            def emit_pvhat_chunk(w, g, jc, eTs):
                """One jc-quarter of the augmented-V PV accumulation.

                Two 128-wide stationaries per head pair (slots at
                disjoint 32-col blocks) interleave into one PSUM tile:
                rows 0-64 = 32-copied denominators, 64-128 = PV."""
                _, _, vnh, _ = views(wts[w])
                if jc == 0:
                    pvh_of[(w, g)] = (
                        pp.tile([P, 512], F32, name=f"pv0{w}{g}",
                                tag="pv0", bufs=1),
                        pp.tile([P, 512], F32, name=f"pv1{w}{g}",
                                tag="pv1", bufs=2))
                pvh = pvh_of[(w, g)]
                if w < NW - 1:
                    qo, qn = 0, 512
                    st, sp_ = (jc == 0), (jc == 3)
                else:
                    qh, jx = jc // 2, jc % 2
                    qo, qn = 256 * qh, 256
                    st, sp_ = (jx == 0), (jx == 1)
                for pair in range(2):
                    for s in range(2):
                        hp = 2 * pair + s
                        nc.tensor.matmul(
                            pvh[pair][:, qo:qo + qn],
                            vnh[:, g, jc, hp, :],
                            eTs[jc][:, 512 * hp + qo:512 * hp + qo + qn],
                            start=st and s == 0, stop=sp_ and s == 1,
                            tile_position=(0, 0))

            mg_of = {}

            def emit_finish(w, g):
                """Normalize + merge LePE per head pair.

                SBUF same-start-partition rule: pair 1's ops must run at
                base 64, but custom-DVE (recip) only works base-0 -> its
                result is realigned to rows 64-128 by a gpsimd copy."""
                _, _, _, lpT = views(wts[w])
                pvh = pvh_of.pop((w, g))
                mg = sp.tile([P, 512], BF16, name=f"mg{w}{g}", tag="mg", bufs=4)
                rbbF = sp.tile([P, 512], F32, name=f"rbb{w}{g}",
                               tag="rbb", bufs=3)
                # pair 0's full chain first: its PSUM tile frees ~1.3us
                # earlier, relieving the next pair's PV-hat WAR stall
                nc.vector.reciprocal_approx_fast(rbbF[0:64, :], pvh[0][0:64, :])
                nc.vector.tensor_tensor(
                    out=mg[0:64, :], in0=pvh[0][64:128, :],
                    in1=rbbF[0:64, :], op=mybir.AluOpType.mult)
                nc.vector.tensor_tensor(
                    out=mg[0:64, :], in0=mg[0:64, :], in1=lpT[0:64, g, :],
                    op=mybir.AluOpType.add)
                rscr = sp.tile([64, 512], F32, name=f"rs{w}{g}",
                               tag="rscr", bufs=2)
                nc.vector.reciprocal_approx_fast(rscr[:], pvh[1][0:64, :])
                nc.vector.tensor_scalar(
                    out=rbbF[64:128, :], in0=rscr[:], scalar1=1.0,
                    scalar2=None, op0=mybir.AluOpType.mult)
                nc.vector.tensor_tensor(
                    out=mg[64:128, :], in0=pvh[1][64:128, :],
                    in1=rbbF[64:128, :], op=mybir.AluOpType.mult)
                nc.vector.tensor_tensor(
                    out=mg[64:128, :], in0=mg[64:128, :], in1=lpT[64:128, g, :],
                    op=mybir.AluOpType.add)
                mg_of[(w, g)] = mg

            ob_of = {}
            pj_of = {}

            def emit_pj_piece(w, t4):
                """Transposed proj: out [c_out, q]. Slot t4 -> (chunk, g).

                pw chunks are the stationaries; mg is the 512-col moving,
                so LDWEIGHTS fully hides and MM count halves. Bias is
                per-partition (c_out) -> a single tensor_scalar."""
                k, g2 = t4 // 2, t4 % 2
                if t4 == 0:
                    ob_of[w] = sp.tile([P, 2, 512], BF16, name=f"ob{w}",
                                       tag="ob", bufs=2)
                if g2 == 0:
                    pj_of[(w, k)] = pp.tile([P, 512], F32, name=f"pj{w}{k}",
                                            tag="pj", bufs=1)
                pjT = pj_of[(w, k)]
                nc.tensor.matmul(pjT[:], pw_sb[:, g2, k, :],
                                 mg_of[(w, g2)][:],
                                 start=(g2 == 0), stop=(g2 == 1))
                if g2 == 1:
                    ob = ob_of[w]
                    nc.vector.tensor_scalar(
                        out=ob[:, k, :], in0=pj_of.pop((w, k))[:],
                        scalar1=pb_sb[:, k:k + 1], scalar2=None,
                        op0=mybir.AluOpType.add)
                    nc.sync.dma_start(out[:][w][k], ob[:, k, :])
                if t4 == 3:
                    ob_of.pop(w)
                    del mg_of[(w, 0)], mg_of[(w, 1)]

            # fine-grained software pipeline: per jc-slot emit this pair's
            # QK+exp, then the PREVIOUS pair's PV-hat quarter, then (during
            # g=1 pairs) one proj piece of the previous window.
            pairs = [(w, g) for w in range(NW) for g in range(2)]
            prev = None
            for w, g in pairs:
                last = (w == NW - 1 and g == 1)
                if g == 0 and w + 1 < NW:   # prefetch next window's blob
                    nwt = sp.tile([P, WCOLS], BF16, name=f"wt{w + 1}",
                                  tag="wt", bufs=3)
                    nc.sync.dma_start(nwt[:], win[:][w + 1])
                    wts[w + 1] = nwt
                eTs = []
                for jc in range(4):
                    eTs.append(emit_bg_exp(w, g, jc))
                    if prev is not None:
                        emit_pvhat_chunk(prev[0], prev[1], jc, prev[2])
                    if last:   # no next slot: run own PV-hat inline
                        emit_pvhat_chunk(w, g, jc, eTs)
                    if g == 1 and w >= 1:
                        emit_pj_piece(w - 1, jc)
                if prev is not None:
                    emit_finish(prev[0], prev[1])
                prev = (w, g, eTs)
            emit_finish(NW - 1, 1)
            for t4 in range(4):
                emit_pj_piece(NW - 1, t4)
    return nc


_CACHE = {}


def _get_nc():
    if "nc" not in _CACHE:
        nc = build_nc()
        nc.finalize()
        _CACHE["nc"] = nc
    return _CACHE["nc"]


def _host_lepe(v_win, conv_w, conv_b):
    """Depthwise 3x3 conv on [B, NW, C, 64, 8] window images (host, fp32).

    Each 64x8 window is zero-padded independently, matching the
    reference's per-window lax.conv on [B*nW, C, Hsp, Wsp]."""
    Bx, nw, C, H, W = v_win.shape
    pad = np.zeros((Bx, nw, C, H + 2, W + 2), np.float32)
    pad[:, :, :, 1:-1, 1:-1] = v_win
    out = np.broadcast_to(
        conv_b[None, None, :, None, None], v_win.shape).copy()
    cw = conv_w.reshape(C, 3, 3)
    for dy in range(3):
        for dx in range(3):
            out += cw[None, None, :, dy, dx, None, None] * \
                pad[:, :, :, dy:dy + H, dx:dx + W]
    return out


def _host_prep(qkv, scale, proj_w, proj_b, conv_w, conv_b):
    """Per-core input maps: all device layouts built host-side."""
    scale_v = float(np.asarray(scale).reshape(-1)[0])
    q_all = np.asarray(qkv[0], np.float32) * scale_v
    k_all = np.asarray(qkv[1], np.float32)
    v_all = np.asarray(qkv[2], np.float32)
    conv_w_h = np.asarray(conv_w, np.float32)
    conv_b_h = np.asarray(conv_b, np.float32)

    # weights (shared across cores). conv bias is folded into the lepe
    # plane itself (host conv adds it), so proj bias is just proj_b.
    # pw stationary: S[ch, g, k, c] = proj_w[k*128+c, g*128+ch]
    pw_h = np.ascontiguousarray(
        np.asarray(proj_w, np.float32).reshape(2, P, 2, P)
        .transpose(3, 2, 0, 1).reshape(P, 2 * DIM)).astype(ml_dtypes.bfloat16)
    pb_h = np.ascontiguousarray(
        np.asarray(proj_b, np.float32).reshape(2, P).T)

    # token reorder: l = h*64 + w*8 + s  ->  window w, t' = s*64 + h
    def to_win(x):
        xw = x.reshape(B, RESO, NW, STRIPE, DIM)          # [b, h, w, s, c]
        return np.ascontiguousarray(xw.transpose(0, 2, 3, 1, 4)).reshape(
            B, NW, WIN, DIM)                               # [b, w, s*64+h, c]

    qw = to_win(q_all)
    kw = to_win(k_all)
    vw = to_win(v_all)

    # lepe: per-window depthwise conv; vw is [b, w, (s h), c]
    v_win = vw.reshape(B, NW, STRIPE, RESO, DIM).transpose(0, 1, 4, 3, 2)
    lepe = _host_lepe(v_win, conv_w_h, conv_b_h)      # [b, w, c, h, s]
    lw = np.ascontiguousarray(lepe.transpose(0, 1, 4, 3, 2)).reshape(
        B, NW, WIN, DIM)                               # [b, w, (s h), c]

    # fused per-window blob [B, NW, P, WCOLS]: bf16 planes for qT/kT/lepeT,
    # fp16 bits for the vn-hat plane (PV runs in fp16 to match the
    # Schraudolph fp16 eT tiles).
    blob = np.zeros((B, NW, P, WCOLS), np.uint16)

    def bf16_bits(x):
        return x.astype(ml_dtypes.bfloat16).view(np.uint16)

    # qT/kT/lepeT: [p = ch within g, g*512 + t']
    for off, src in ((O_QT, qw), (O_KT, kw), (O_LP, lw)):
        t = src.transpose(0, 1, 3, 2).reshape(B, NW, 2, P, WIN)
        blob[:, :, :, off:off + 1024] = bf16_bits(
            t.transpose(0, 1, 3, 2, 4).reshape(B, NW, P, 1024))

    # vn-hat stationaries: [p = key within jc chunk, (g, jc, hp, 128)]
    # per slot s = hp%2: ones at cols [32s, 32s+32), v at [64+32s, +32).
    vn = np.zeros((B, NW, 2, 4, 4, P, P), np.float16)  # [b,w,g,jc,hp,key,col]
    vw4 = vw.reshape(B, NW, 4, P, NH, HD)              # [b,w,jc,key,head,ch]
    for g in range(2):
        for hp in range(4):
            s = hp % 2
            vn[:, :, g, :, hp, :, 32 * s:32 * s + 32] = 1.0
            vn[:, :, g, :, hp, :, 64 + 32 * s:96 + 32 * s] = \
                vw4[:, :, :, :, 4 * g + hp, :]
    blob[:, :, :, O_VN:O_VN + 4096] = vn.transpose(
        0, 1, 5, 2, 3, 4, 6).reshape(B, NW, P, 4096).view(np.uint16)
    blob_bf = blob.view(ml_dtypes.bfloat16)

    in_maps = []
    for b in range(B):
        in_maps.append({
            "win": np.ascontiguousarray(blob_bf[b]),
            "pw": pw_h, "pb": pb_h,
        })
    return in_maps


LAST_RESULTS = None


def kernel(qkv, scale, proj_w, proj_b, conv_w, conv_b):
    global LAST_RESULTS
    from concourse.bass_utils import run_bass_kernel_spmd
    nc = _get_nc()
    in_maps = _host_prep(qkv, scale, proj_w, proj_b, conv_w, conv_b)
    res = run_bass_kernel_spmd(nc, in_maps, core_ids=list(range(B)))
    LAST_RESULTS = res
    outs = []
    for b in range(B):
        o = np.asarray(res.results[b]["out"]).astype(np.float32)
        # device layout: [w, k, c, q] with t' = q = s*64 + h
        o = o.transpose(0, 3, 1, 2).reshape(NW, WIN, DIM)    # [w, t', c]
        o = o.reshape(NW, STRIPE, RESO, DIM)                 # [w, s, h, c]
        o = o.transpose(2, 0, 1, 3).reshape(L, DIM)          # [h*64+w*8+s, c]
        outs.append(o)
    return np.stack(outs, axis=0)


# revision 25
# speedup vs baseline: 1.0465x; 1.0465x over previous
"""CSwin vertical-stripe window attention (sparse_attention) on 8 TRN2 cores.

Sharding: data-parallel over batch B=8 (one image per NeuronCore). No
collectives. Per-core kernel computes windowed attention + output
projection for one [4096, 256] image; the tiny LePE depthwise 3x3 conv
(0.7% of FLOPs) is folded host-side into a per-window additive plane.

v7 design (softmax-denominator matmuls eliminated via augmented V):
 - The PV stationary for each head pair is widened to 128 cols:
   [ones(32) | ones(32) | v_even(32) | v_odd(32)] with disjoint 32-col
   blocks per slot, so ONE set of PV matmuls yields both the attention
   output (PSUM rows 64-128) and 32 duplicated copies of each softmax
   denominator (rows 0-64) -- the v6 SM matmuls (1/3 of all PE columns)
   are gone at zero extra PE cost (PE time ~ moving columns only).
 - finish: one reciprocal_approx_fast [64,512] on the den rows (base-0,
   custom-DVE ops require base-0 partition addressing) + one cross-base
   tensor_tensor multiply + one lepe add per head pair.
 - QK^T bf16, 4-head row-packed, split across TWO 2-bank PSUM tiles
   (heads 01 -> bigA, heads 23 -> bigB); fine-grained software pipeline
   emits, per jc-slot, this pair's QK+exp then the previous pair's
   PV-hat quarter and a proj piece, so no engine queue head blocks.
 - exp: Scalar-engine ACTIVATE for most tiles; some halves use a
   single-instruction DVE Schraudolph (fp16-bit trick: round(x*a+b) as
   int16 IS the fp16 exp, ~3% max rel err, bias cancels in softmax).
 - Window 7 (shifted stripes) is block-diagonal: masked quadrants are
   never computed (N=256 matmuls, strided Exp).
 - proj bias added by the DVE PSUM-evacuation op; output bf16
   window-major contiguous, host un-permutes.
"""
import numpy as np
import ml_dtypes

import concourse.bass as bass
import concourse.bacc as bacc
import concourse.mybir as mybir
import concourse.tile as tile

RESO, STRIPE, DIM, NH, HD = 64, 8, 256, 8, 32
B, L, WIN, NW = 8, RESO * RESO, RESO * STRIPE, RESO // STRIPE
P = 128
F32, BF16 = mybir.dt.float32, mybir.dt.bfloat16
F16, I16 = mybir.dt.float16, mybir.dt.int16

# fused per-window input blob offsets (16-bit elements, per partition)
O_QT, O_KT, O_VN, O_LP = 0, 1024, 2048, 6144
WCOLS = 7168

Exp = mybir.ActivationFunctionType.Exp
# Schraudolph fp16 exp: fp16_bits(e^x) ~ round(x * SCH_A + SCH_B)
SCH_A, SCH_B = 1477.3197218702985, 15315.5


def _dve_exp_half(w, g, jc, h0):
    """Which exp halves run on the Vector engine instead of Scalar.

    B-halves only: the offload's pipeline win is freeing bigB early so
    the next QK group unblocks; A-half offloads only added Vector-engine
    contention."""
    return w != NW - 1 and jc == 1 and h0 == 2


def build_nc():
    nc = bacc.Bacc("TRN2", target_bir_lowering=False, debug=False)
    win = nc.declare_dram_parameter("win", [NW, P, WCOLS], BF16, isOutput=False)
    pw = nc.declare_dram_parameter("pw", [P, 2 * DIM], BF16, isOutput=False)
    pb = nc.declare_dram_parameter("pb", [P, 2], F32, isOutput=False)
    # transposed output: [w, c-chunk, c, q]; host un-permutes
    out = nc.declare_dram_parameter("out", [NW, 2, P, 512], BF16,
                                    isOutput=True)

    with tile.TileContext(nc) as tc:
        with tc.tile_pool(name="const", bufs=1) as cp, \
             tc.tile_pool(name="sb", bufs=1) as sp, \
             tc.tile_pool(name="ps", bufs=1, space="PSUM") as pp:
            # ---- first window's qk plane goes out before anything else ----
            wts = {}
            # warmup: ramp PE pstate + pull the ACT table while DMAs run
            wu = cp.tile([P, 128], BF16, name="wu")
            nc.vector.memset(wu[:], 0.0)
            wups = pp.tile([P, 512], F32, name="wups", tag="pj", bufs=2)
            nc.tensor.matmul(wups[:, 0:128], wu[:], wu[:],
                             start=True, stop=True)
            wue = cp.tile([P, 128], F16, name="wue")
            nc.scalar.activation(wue[:], wups[:, 0:128], Exp,
                                 bias=0.0, scale=1.0)

            wt0 = sp.tile([P, WCOLS], BF16, name="wt0", tag="wt", bufs=3)
            nc.sync.dma_start(wt0[:, :512], win[:][0][:, :512])
            nc.sync.dma_start(wt0[:, 1024:1536], win[:][0][:, 1024:1536])
            nc.sync.dma_start(wt0[:, 512:1024], win[:][0][:, 512:1024])
            nc.sync.dma_start(wt0[:, 1536:2048], win[:][0][:, 1536:2048])
            nc.sync.dma_start(wt0[:, 2048:4096], win[:][0][:, 2048:4096])
            nc.sync.dma_start(wt0[:, 4096:], win[:][0][:, 4096:])
            wts[0] = wt0

            # ---- constants ----
            pw_sb = cp.tile([P, 2, 2, P], BF16, name="pw_sb")
            nc.sync.dma_start(pw_sb[:], pw[:].rearrange(
                "p (g k c) -> p g k c", g=2, k=2))
            pb_sb = cp.tile([P, 2], F32, name="pb_sb")
            nc.sync.dma_start(pb_sb[:], pb[:])

            def views(wt):
                return (
                    wt[:, O_QT:O_QT + 1024].rearrange("p (g q) -> p g q", g=2),
                    wt[:, O_KT:O_KT + 1024].rearrange("p (g q) -> p g q", g=2),
                    wt[:, O_VN:O_VN + 4096].bitcast(F16).rearrange(
                        "p (g j h c) -> p g j h c", g=2, j=4, h=4),
                    wt[:, O_LP:O_LP + 1024].rearrange("p (g q) -> p g q", g=2),
                )

            def emit_bg_exp(w, g, jc):
                """One QK jc-quarter (4 row-packed bf16 MMs into the A/B
                PSUM pair) followed by its two exps (Scalar or Vector)."""
                qT, kT, _, _ = views(wts[w])
                bigA = pp.tile([P, 1024], F32, name=f"bA{w}{g}{jc}",
                               tag="bigA", bufs=1)
                bigB = pp.tile([P, 1024], F32, name=f"bB{w}{g}{jc}",
                               tag="bigB", bufs=1)
                eT = sp.tile([P, 2048], F16, name=f"eT{w}{g}{jc}",
                             tag="eT", bufs=12)
                halves = ((bigA, 0), (bigB, 2))
                if w < NW - 1:
                    for big, h0 in halves:
                        for hx in range(2):
                            hp = h0 + hx
                            nc.tensor.matmul(
                                big[:, 512 * hx:512 * (hx + 1)],
                                kT[32 * hp:32 * hp + 32, g,
                                   P * jc:P * (jc + 1)],
                                qT[32 * hp:32 * hp + 32, g, :],
                                start=True, stop=True,
                                tile_position=(32 * hp, 0))
                    for big, h0 in halves:
                        ev = eT[:, 1024 * (h0 // 2):1024 * (h0 // 2) + 1024]
                        if _dve_exp_half(w, g, jc, h0):
                            nc.vector.tensor_scalar(
                                out=ev.bitcast(I16), in0=big[:],
                                scalar1=SCH_A, scalar2=SCH_B,
                                op0=mybir.AluOpType.mult,
                                op1=mybir.AluOpType.add)
                        else:
                            nc.scalar.activation(ev, big[:], Exp,
                                                 bias=0.0, scale=1.0)
                else:
                    # shifted window: block-diagonal mask. keys of
                    # quarter jc only see queries qo..qo+256.
                    qo = 0 if jc < 2 else 256
                    for big, h0 in halves:
                        for hx in range(2):
                            hp = h0 + hx
                            nc.tensor.matmul(
                                big[:, 512 * hx + qo:512 * hx + qo + 256],
                                kT[32 * hp:32 * hp + 32, g,
                                   P * jc:P * (jc + 1)],
                                qT[32 * hp:32 * hp + 32, g, qo:qo + 256],
                                start=True, stop=True,
                                tile_position=(32 * hp, 0))
                    for big, h0 in halves:
                        bv = big[:].rearrange(
                            "p (h q) -> p h q", h=2)[:, :, qo:qo + 256]
                        ev = eT[:, 1024 * (h0 // 2):
                                1024 * (h0 // 2) + 1024].rearrange(
                            "p (h q) -> p h q", h=2)[:, :, qo:qo + 256]
                        nc.scalar.activation(ev, bv, Exp,
                                             bias=0.0, scale=1.0)
                return eT

            pvh_of = {}

            def emit_pvhat_chunk(w, g, jc, eTs):
                """One jc-quarter of the augmented-V PV accumulation.

                Two 128-wide stationaries per head pair (slots at
                disjoint 32-col blocks) interleave into one PSUM tile:
                rows 0-64 = 32-copied denominators, 64-128 = PV."""
                _, _, vnh, _ = views(wts[w])
                if jc == 0:
                    pvh_of[(w, g)] = (
                        pp.tile([P, 512], F32, name=f"pv0{w}{g}",
                                tag="pv0", bufs=1),
                        pp.tile([P, 512], F32, name=f"pv1{w}{g}",
                                tag="pv1", bufs=1))
                pvh = pvh_of[(w, g)]
                if w < NW - 1:
                    qo, qn = 0, 512
                    st, sp_ = (jc == 0), (jc == 3)
                else:
                    qh, jx = jc // 2, jc % 2
                    qo, qn = 256 * qh, 256
                    st, sp_ = (jx == 0), (jx == 1)
                for pair in range(2):
                    for s in range(2):
                        hp = 2 * pair + s
                        nc.tensor.matmul(
                            pvh[pair][:, qo:qo + qn],
                            vnh[:, g, jc, hp, :],
                            eTs[jc][:, 512 * hp + qo:512 * hp + qo + qn],
                            start=st and s == 0, stop=sp_ and s == 1,
                            tile_position=(0, 0))

            mg_of = {}

            def emit_finish(w, g):
                """Normalize + merge LePE per head pair.

                SBUF same-start-partition rule: pair 1's ops must run at
                base 64, but custom-DVE (recip) only works base-0 -> its
                result is realigned to rows 64-128 by a gpsimd copy."""
                _, _, _, lpT = views(wts[w])
                pvh = pvh_of.pop((w, g))
                mg = sp.tile([P, 512], BF16, name=f"mg{w}{g}", tag="mg", bufs=4)
                rbbF = sp.tile([P, 512], F32, name=f"rbb{w}{g}",
                               tag="rbb", bufs=3)
                # pair 0's full chain first: its PSUM tile frees ~1.3us
                # earlier, relieving the next pair's PV-hat WAR stall
                nc.vector.reciprocal_approx_fast(rbbF[0:64, :], pvh[0][0:64, :])
                nc.vector.tensor_tensor(
                    out=mg[0:64, :], in0=pvh[0][64:128, :],
                    in1=rbbF[0:64, :], op=mybir.AluOpType.mult)
                nc.vector.tensor_tensor(
                    out=mg[0:64, :], in0=mg[0:64, :], in1=lpT[0:64, g, :],
                    op=mybir.AluOpType.add)
                rscr = sp.tile([64, 512], F32, name=f"rs{w}{g}",
                               tag="rscr", bufs=2)
                nc.vector.reciprocal_approx_fast(rscr[:], pvh[1][0:64, :])
                nc.vector.tensor_scalar(
                    out=rbbF[64:128, :], in0=rscr[:], scalar1=1.0,
                    scalar2=None, op0=mybir.AluOpType.mult)
                nc.vector.tensor_tensor(
                    out=mg[64:128, :], in0=pvh[1][64:128, :],
                    in1=rbbF[64:128, :], op=mybir.AluOpType.mult)
                nc.vector.tensor_tensor(
                    out=mg[64:128, :], in0=mg[64:128, :], in1=lpT[64:128, g, :],
                    op=mybir.AluOpType.add)
                mg_of[(w, g)] = mg

            ob_of = {}
            pj_of = {}

            def emit_pj_piece(w, t4):
                """Transposed proj: out [c_out, q]. Slot t4 -> (chunk, g).

                pw chunks are the stationaries; mg is the 512-col moving,
                so LDWEIGHTS fully hides and MM count halves. Bias is
                per-partition (c_out) -> a single tensor_scalar."""
                k, g2 = t4 // 2, t4 % 2
                if t4 == 0:
                    ob_of[w] = sp.tile([P, 2, 512], BF16, name=f"ob{w}",
                                       tag="ob", bufs=2)
                if g2 == 0:
                    pj_of[(w, k)] = pp.tile([P, 512], F32, name=f"pj{w}{k}",
                                            tag="pj", bufs=2)
                pjT = pj_of[(w, k)]
                nc.tensor.matmul(pjT[:], pw_sb[:, g2, k, :],
                                 mg_of[(w, g2)][:],
                                 start=(g2 == 0), stop=(g2 == 1))
                if g2 == 1:
                    ob = ob_of[w]
                    nc.vector.tensor_scalar(
                        out=ob[:, k, :], in0=pj_of.pop((w, k))[:],
                        scalar1=pb_sb[:, k:k + 1], scalar2=None,
                        op0=mybir.AluOpType.add)
                    nc.sync.dma_start(out[:][w][k], ob[:, k, :])
                if t4 == 3:
                    ob_of.pop(w)
                    del mg_of[(w, 0)], mg_of[(w, 1)]

            # fine-grained software pipeline: per jc-slot emit this pair's
            # QK+exp, then the PREVIOUS pair's PV-hat quarter, then (during
            # g=1 pairs) one proj piece of the previous window.
            pairs = [(w, g) for w in range(NW) for g in range(2)]
            prev = None
            for w, g in pairs:
                last = (w == NW - 1 and g == 1)
                if g == 0 and w + 1 < NW:   # prefetch next window's blob
                    nwt = sp.tile([P, WCOLS], BF16, name=f"wt{w + 1}",
                                  tag="wt", bufs=3)
                    nc.sync.dma_start(nwt[:], win[:][w + 1])
                    wts[w + 1] = nwt
                eTs = []
                for jc in range(4):
                    eTs.append(emit_bg_exp(w, g, jc))
                    if prev is not None:
                        emit_pvhat_chunk(prev[0], prev[1], jc, prev[2])
                    if last:   # no next slot: run own PV-hat inline
                        emit_pvhat_chunk(w, g, jc, eTs)
                    if g == 1 and w >= 1:
                        emit_pj_piece(w - 1, jc)
                if prev is not None:
                    emit_finish(prev[0], prev[1])
                prev = (w, g, eTs)
            emit_finish(NW - 1, 1)
            for t4 in range(4):
                emit_pj_piece(NW - 1, t4)
    return nc


_CACHE = {}


def _get_nc():
    if "nc" not in _CACHE:
        nc = build_nc()
        nc.finalize()
        _CACHE["nc"] = nc
    return _CACHE["nc"]


def _host_lepe(v_win, conv_w, conv_b):
    """Depthwise 3x3 conv on [B, NW, C, 64, 8] window images (host, fp32).

    Each 64x8 window is zero-padded independently, matching the
    reference's per-window lax.conv on [B*nW, C, Hsp, Wsp]."""
    Bx, nw, C, H, W = v_win.shape
    pad = np.zeros((Bx, nw, C, H + 2, W + 2), np.float32)
    pad[:, :, :, 1:-1, 1:-1] = v_win
    out = np.broadcast_to(
        conv_b[None, None, :, None, None], v_win.shape).copy()
    cw = conv_w.reshape(C, 3, 3)
    for dy in range(3):
        for dx in range(3):
            out += cw[None, None, :, dy, dx, None, None] * \
                pad[:, :, :, dy:dy + H, dx:dx + W]
    return out


def _host_prep(qkv, scale, proj_w, proj_b, conv_w, conv_b):
    """Per-core input maps: all device layouts built host-side."""
    scale_v = float(np.asarray(scale).reshape(-1)[0])
    q_all = np.asarray(qkv[0], np.float32) * scale_v
    k_all = np.asarray(qkv[1], np.float32)
    v_all = np.asarray(qkv[2], np.float32)
    conv_w_h = np.asarray(conv_w, np.float32)
    conv_b_h = np.asarray(conv_b, np.float32)

    # weights (shared across cores). conv bias is folded into the lepe
    # plane itself (host conv adds it), so proj bias is just proj_b.
    # pw stationary: S[ch, g, k, c] = proj_w[k*128+c, g*128+ch]
    pw_h = np.ascontiguousarray(
        np.asarray(proj_w, np.float32).reshape(2, P, 2, P)
        .transpose(3, 2, 0, 1).reshape(P, 2 * DIM)).astype(ml_dtypes.bfloat16)
    pb_h = np.ascontiguousarray(
        np.asarray(proj_b, np.float32).reshape(2, P).T)

    # token reorder: l = h*64 + w*8 + s  ->  window w, t' = s*64 + h
    def to_win(x):
        xw = x.reshape(B, RESO, NW, STRIPE, DIM)          # [b, h, w, s, c]
        return np.ascontiguousarray(xw.transpose(0, 2, 3, 1, 4)).reshape(
            B, NW, WIN, DIM)                               # [b, w, s*64+h, c]

    qw = to_win(q_all)
    kw = to_win(k_all)
    vw = to_win(v_all)

    # lepe: per-window depthwise conv; vw is [b, w, (s h), c]
    v_win = vw.reshape(B, NW, STRIPE, RESO, DIM).transpose(0, 1, 4, 3, 2)
    lepe = _host_lepe(v_win, conv_w_h, conv_b_h)      # [b, w, c, h, s]
    lw = np.ascontiguousarray(lepe.transpose(0, 1, 4, 3, 2)).reshape(
        B, NW, WIN, DIM)                               # [b, w, (s h), c]

    # fused per-window blob [B, NW, P, WCOLS]: bf16 planes for qT/kT/lepeT,
    # fp16 bits for the vn-hat plane (PV runs in fp16 to match the
    # Schraudolph fp16 eT tiles).
    blob = np.zeros((B, NW, P, WCOLS), np.uint16)

    def bf16_bits(x):
        return x.astype(ml_dtypes.bfloat16).view(np.uint16)

    # qT/kT/lepeT: [p = ch within g, g*512 + t']
    for off, src in ((O_QT, qw), (O_KT, kw), (O_LP, lw)):
        t = src.transpose(0, 1, 3, 2).reshape(B, NW, 2, P, WIN)
        blob[:, :, :, off:off + 1024] = bf16_bits(
            t.transpose(0, 1, 3, 2, 4).reshape(B, NW, P, 1024))

    # vn-hat stationaries: [p = key within jc chunk, (g, jc, hp, 128)]
    # per slot s = hp%2: ones at cols [32s, 32s+32), v at [64+32s, +32).
    vn = np.zeros((B, NW, 2, 4, 4, P, P), np.float16)  # [b,w,g,jc,hp,key,col]
    vw4 = vw.reshape(B, NW, 4, P, NH, HD)              # [b,w,jc,key,head,ch]
    for g in range(2):
        for hp in range(4):
            s = hp % 2
            vn[:, :, g, :, hp, :, 32 * s:32 * s + 32] = 1.0
            vn[:, :, g, :, hp, :, 64 + 32 * s:96 + 32 * s] = \
                vw4[:, :, :, :, 4 * g + hp, :]
    blob[:, :, :, O_VN:O_VN + 4096] = vn.transpose(
        0, 1, 5, 2, 3, 4, 6).reshape(B, NW, P, 4096).view(np.uint16)
    blob_bf = blob.view(ml_dtypes.bfloat16)

    in_maps = []
    for b in range(B):
        in_maps.append({
            "win": np.ascontiguousarray(blob_bf[b]),
            "pw": pw_h, "pb": pb_h,
        })
    return in_maps


LAST_RESULTS = None


def kernel(qkv, scale, proj_w, proj_b, conv_w, conv_b):
    global LAST_RESULTS
    from concourse.bass_utils import run_bass_kernel_spmd
    nc = _get_nc()
    in_maps = _host_prep(qkv, scale, proj_w, proj_b, conv_w, conv_b)
    res = run_bass_kernel_spmd(nc, in_maps, core_ids=list(range(B)))
    LAST_RESULTS = res
    outs = []
    for b in range(B):
        o = np.asarray(res.results[b]["out"]).astype(np.float32)
        # device layout: [w, k, c, q] with t' = q = s*64 + h
        o = o.transpose(0, 3, 1, 2).reshape(NW, WIN, DIM)    # [w, t', c]
        o = o.reshape(NW, STRIPE, RESO, DIM)                 # [w, s, h, c]
        o = o.transpose(2, 0, 1, 3).reshape(L, DIM)          # [h*64+w*8+s, c]
        outs.append(o)
    return np.stack(outs, axis=0)


# revision 29
# speedup vs baseline: 1.0563x; 1.0093x over previous
"""CSwin vertical-stripe window attention (sparse_attention) on 8 TRN2 cores.

Sharding: data-parallel over batch B=8 (one image per NeuronCore). No
collectives. Per-core kernel computes windowed attention + output
projection for one [4096, 256] image; the tiny LePE depthwise 3x3 conv
(0.7% of FLOPs) is folded host-side into a per-window additive plane.

v7 design (softmax-denominator matmuls eliminated via augmented V):
 - The PV stationary for each head pair is widened to 128 cols:
   [ones(32) | ones(32) | v_even(32) | v_odd(32)] with disjoint 32-col
   blocks per slot, so ONE set of PV matmuls yields both the attention
   output (PSUM rows 64-128) and 32 duplicated copies of each softmax
   denominator (rows 0-64) -- the v6 SM matmuls (1/3 of all PE columns)
   are gone at zero extra PE cost (PE time ~ moving columns only).
 - finish: one reciprocal_approx_fast [64,512] on the den rows (base-0,
   custom-DVE ops require base-0 partition addressing) + one cross-base
   tensor_tensor multiply + one lepe add per head pair.
 - QK^T bf16, 4-head row-packed, split across TWO 2-bank PSUM tiles
   (heads 01 -> bigA, heads 23 -> bigB); fine-grained software pipeline
   emits, per jc-slot, this pair's QK+exp then the previous pair's
   PV-hat quarter and a proj piece, so no engine queue head blocks.
 - exp: Scalar-engine ACTIVATE for most tiles; some halves use a
   single-instruction DVE Schraudolph (fp16-bit trick: round(x*a+b) as
   int16 IS the fp16 exp, ~3% max rel err, bias cancels in softmax).
 - Window 7 (shifted stripes) is block-diagonal: masked quadrants are
   never computed (N=256 matmuls, strided Exp).
 - proj bias added by the DVE PSUM-evacuation op; output bf16
   window-major contiguous, host un-permutes.
"""
import numpy as np
import ml_dtypes

import concourse.bass as bass
import concourse.bacc as bacc
import concourse.mybir as mybir
import concourse.tile as tile

RESO, STRIPE, DIM, NH, HD = 64, 8, 256, 8, 32
B, L, WIN, NW = 8, RESO * RESO, RESO * STRIPE, RESO // STRIPE
P = 128
F32, BF16 = mybir.dt.float32, mybir.dt.bfloat16
F16, I16 = mybir.dt.float16, mybir.dt.int16

# fused per-window input blob offsets (16-bit elements, per partition)
O_QT, O_KT, O_VN, O_LP = 0, 1024, 2048, 6144
WCOLS = 7168

Exp = mybir.ActivationFunctionType.Exp
# Schraudolph fp16 exp: fp16_bits(e^x) ~ round(x * SCH_A + SCH_B)
SCH_A, SCH_B = 1477.3197218702985, 15315.5


def _dve_exp_half(w, g, jc, h0):
    """Which exp halves run on the Vector engine instead of Scalar.

    B-halves only: the offload's pipeline win is freeing bigB early so
    the next QK group unblocks; A-half offloads only added Vector-engine
    contention."""
    return w != NW - 1 and jc == 1 and h0 == 2


def build_nc():
    nc = bacc.Bacc("TRN2", target_bir_lowering=False, debug=False)
    win = nc.declare_dram_parameter("win", [NW, P, WCOLS], BF16, isOutput=False)
    pw = nc.declare_dram_parameter("pw", [P, 2 * DIM], BF16, isOutput=False)
    pb = nc.declare_dram_parameter("pb", [P, 2], F32, isOutput=False)
    # transposed output: [w, c-chunk, c, q]; host un-permutes
    out = nc.declare_dram_parameter("out", [NW, 2, P, 512], BF16,
                                    isOutput=True)

    with tile.TileContext(nc) as tc:
        with tc.tile_pool(name="const", bufs=1) as cp, \
             tc.tile_pool(name="sb", bufs=1) as sp, \
             tc.tile_pool(name="ps", bufs=1, space="PSUM") as pp:
            # ---- first window's qk plane goes out before anything else ----
            wts = {}
            # warmup: ramp PE pstate + pull the ACT table while DMAs run
            wu = cp.tile([P, 128], BF16, name="wu")
            nc.vector.memset(wu[:], 0.0)
            wups = pp.tile([P, 512], F32, name="wups", tag="pj", bufs=2)
            nc.tensor.matmul(wups[:, 0:128], wu[:], wu[:],
                             start=True, stop=True)
            wue = cp.tile([P, 128], F16, name="wue")
            nc.scalar.activation(wue[:], wups[:, 0:128], Exp,
                                 bias=0.0, scale=1.0)

            wt0 = sp.tile([P, WCOLS], BF16, name="wt0", tag="wt", bufs=3)
            nc.sync.dma_start(wt0[:, :512], win[:][0][:, :512])
            nc.sync.dma_start(wt0[:, 1024:1536], win[:][0][:, 1024:1536])
            nc.sync.dma_start(wt0[:, 512:1024], win[:][0][:, 512:1024])
            nc.sync.dma_start(wt0[:, 1536:2048], win[:][0][:, 1536:2048])
            nc.sync.dma_start(wt0[:, 2048:4096], win[:][0][:, 2048:4096])
            nc.sync.dma_start(wt0[:, 4096:], win[:][0][:, 4096:])
            wts[0] = wt0

            # ---- constants ----
            pw_sb = cp.tile([P, 2, 2, P], BF16, name="pw_sb")
            nc.sync.dma_start(pw_sb[:], pw[:].rearrange(
                "p (g k c) -> p g k c", g=2, k=2))
            pb_sb = cp.tile([P, 2], F32, name="pb_sb")
            nc.sync.dma_start(pb_sb[:], pb[:])

            def views(wt):
                return (
                    wt[:, O_QT:O_QT + 1024].rearrange("p (g q) -> p g q", g=2),
                    wt[:, O_KT:O_KT + 1024].rearrange("p (g q) -> p g q", g=2),
                    wt[:, O_VN:O_VN + 4096].bitcast(F16).rearrange(
                        "p (g j h c) -> p g j h c", g=2, j=4, h=4),
                    wt[:, O_LP:O_LP + 1024].rearrange("p (g q) -> p g q", g=2),
                )

            def emit_bg_exp(w, g, jc):
                """One QK jc-quarter (4 row-packed bf16 MMs into the A/B
                PSUM pair) followed by its two exps (Scalar or Vector)."""
                qT, kT, _, _ = views(wts[w])
                bigA = pp.tile([P, 1024], F32, name=f"bA{w}{g}{jc}",
                               tag="bigA", bufs=1)
                bigB = pp.tile([P, 1024], F32, name=f"bB{w}{g}{jc}",
                               tag="bigB", bufs=1)
                eT = sp.tile([P, 2048], F16, name=f"eT{w}{g}{jc}",
                             tag="eT", bufs=12)
                halves = ((bigA, 0), (bigB, 2))
                if w < NW - 1:
                    for big, h0 in halves:
                        for hx in range(2):
                            hp = h0 + hx
                            nc.tensor.matmul(
                                big[:, 512 * hx:512 * (hx + 1)],
                                kT[32 * hp:32 * hp + 32, g,
                                   P * jc:P * (jc + 1)],
                                qT[32 * hp:32 * hp + 32, g, :],
                                start=True, stop=True,
                                tile_position=(32 * hp, 0))
                    for big, h0 in halves:
                        ev = eT[:, 1024 * (h0 // 2):1024 * (h0 // 2) + 1024]
                        if _dve_exp_half(w, g, jc, h0):
                            nc.vector.tensor_scalar(
                                out=ev.bitcast(I16), in0=big[:],
                                scalar1=SCH_A, scalar2=SCH_B,
                                op0=mybir.AluOpType.mult,
                                op1=mybir.AluOpType.add)
                        else:
                            nc.scalar.activation(ev, big[:], Exp,
                                                 bias=0.0, scale=1.0)
                else:
                    # shifted window: block-diagonal mask. keys of
                    # quarter jc only see queries qo..qo+256.
                    qo = 0 if jc < 2 else 256
                    for big, h0 in halves:
                        for hx in range(2):
                            hp = h0 + hx
                            nc.tensor.matmul(
                                big[:, 512 * hx + qo:512 * hx + qo + 256],
                                kT[32 * hp:32 * hp + 32, g,
                                   P * jc:P * (jc + 1)],
                                qT[32 * hp:32 * hp + 32, g, qo:qo + 256],
                                start=True, stop=True,
                                tile_position=(32 * hp, 0))
                    for big, h0 in halves:
                        bv = big[:].rearrange(
                            "p (h q) -> p h q", h=2)[:, :, qo:qo + 256]
                        ev = eT[:, 1024 * (h0 // 2):
                                1024 * (h0 // 2) + 1024].rearrange(
                            "p (h q) -> p h q", h=2)[:, :, qo:qo + 256]
                        nc.scalar.activation(ev, bv, Exp,
                                             bias=0.0, scale=1.0)
                return eT

            pvh_of = {}

            def emit_pvhat_chunk(w, g, jc, eTs):
                """One jc-quarter of the augmented-V PV accumulation.

                Two 128-wide stationaries per head pair (slots at
                disjoint 32-col blocks) interleave into one PSUM tile:
                rows 0-64 = 32-copied denominators, 64-128 = PV."""
                _, _, vnh, _ = views(wts[w])
                if jc == 0:
                    pvh_of[(w, g)] = (
                        pp.tile([P, 512], F32, name=f"pv0{w}{g}",
                                tag="pv0", bufs=1),
                        pp.tile([P, 512], F32, name=f"pv1{w}{g}",
                                tag="pv1", bufs=1))
                pvh = pvh_of[(w, g)]
                if w < NW - 1:
                    qo, qn = 0, 512
                    st, sp_ = (jc == 0), (jc == 3)
                else:
                    qh, jx = jc // 2, jc % 2
                    qo, qn = 256 * qh, 256
                    st, sp_ = (jx == 0), (jx == 1)
                for pair in range(2):
                    for s in range(2):
                        hp = 2 * pair + s
                        nc.tensor.matmul(
                            pvh[pair][:, qo:qo + qn],
                            vnh[:, g, jc, hp, :],
                            eTs[jc][:, 512 * hp + qo:512 * hp + qo + qn],
                            start=st and s == 0, stop=sp_ and s == 1,
                            tile_position=(0, 0))

            mg_of = {}

            def emit_finish(w, g):
                """Normalize + merge LePE per head pair.

                SBUF same-start-partition rule: pair 1's ops must run at
                base 64, but custom-DVE (recip) only works base-0 -> its
                result is realigned to rows 64-128 by a gpsimd copy."""
                _, _, _, lpT = views(wts[w])
                pvh = pvh_of.pop((w, g))
                mg = sp.tile([P, 512], BF16, name=f"mg{w}{g}", tag="mg", bufs=4)
                rbbF = sp.tile([P, 512], F32, name=f"rbb{w}{g}",
                               tag="rbb", bufs=3)
                # pair 0's full chain first: its PSUM tile frees ~1.3us
                # earlier, relieving the next pair's PV-hat WAR stall
                nc.vector.reciprocal_approx_fast(rbbF[0:64, :], pvh[0][0:64, :])
                nc.vector.tensor_tensor(
                    out=mg[0:64, :], in0=pvh[0][64:128, :],
                    in1=rbbF[0:64, :], op=mybir.AluOpType.mult)
                nc.vector.tensor_tensor(
                    out=mg[0:64, :], in0=mg[0:64, :], in1=lpT[0:64, g, :],
                    op=mybir.AluOpType.add)
                rscr = sp.tile([64, 512], F32, name=f"rs{w}{g}",
                               tag="rscr", bufs=2)
                nc.vector.reciprocal_approx_fast(rscr[:], pvh[1][0:64, :])
                nc.vector.tensor_scalar(
                    out=rbbF[64:128, :], in0=rscr[:], scalar1=1.0,
                    scalar2=None, op0=mybir.AluOpType.mult)
                nc.vector.tensor_tensor(
                    out=mg[64:128, :], in0=pvh[1][64:128, :],
                    in1=rbbF[64:128, :], op=mybir.AluOpType.mult)
                nc.vector.tensor_tensor(
                    out=mg[64:128, :], in0=mg[64:128, :], in1=lpT[64:128, g, :],
                    op=mybir.AluOpType.add)
                mg_of[(w, g)] = mg

            ob_of = {}
            pj_of = {}

            def emit_pj_piece(w, t4):
                """Transposed proj: out [c_out, q]. Slot t4 -> (chunk, g).

                pw chunks are the stationaries; mg is the 512-col moving,
                so LDWEIGHTS fully hides and MM count halves. Bias is
                per-partition (c_out) -> a single tensor_scalar."""
                k, g2 = t4 // 2, t4 % 2
                if t4 == 0:
                    ob_of[w] = sp.tile([P, 2, 512], BF16, name=f"ob{w}",
                                       tag="ob", bufs=2)
                if g2 == 0:
                    pj_of[(w, k)] = pp.tile([P, 512], F32, name=f"pj{w}{k}",
                                            tag="pj", bufs=2)
                pjT = pj_of[(w, k)]
                nc.tensor.matmul(pjT[:], pw_sb[:, g2, k, :],
                                 mg_of[(w, g2)][:],
                                 start=(g2 == 0), stop=(g2 == 1))
                if g2 == 1:
                    ob = ob_of[w]
                    nc.vector.tensor_scalar(
                        out=ob[:, k, :], in0=pj_of.pop((w, k))[:],
                        scalar1=pb_sb[:, k:k + 1], scalar2=None,
                        op0=mybir.AluOpType.add)
                    nc.sync.dma_start(out[:][w][k], ob[:, k, :])
                if t4 == 3:
                    ob_of.pop(w)
                    del mg_of[(w, 0)], mg_of[(w, 1)]

            # fine-grained software pipeline: per jc-slot emit this pair's
            # QK+exp, then the PREVIOUS pair's PV-hat quarter, then (during
            # g=1 pairs) one proj piece of the previous window.
            pairs = [(w, g) for w in range(NW) for g in range(2)]
            prev = None
            for w, g in pairs:
                last = (w == NW - 1 and g == 1)
                if g == 0 and w + 1 < NW:   # prefetch next window's blob
                    nwt = sp.tile([P, WCOLS], BF16, name=f"wt{w + 1}",
                                  tag="wt", bufs=3)
                    nc.sync.dma_start(nwt[:], win[:][w + 1])
                    wts[w + 1] = nwt
                eTs = []
                for jc in range(4):
                    eTs.append(emit_bg_exp(w, g, jc))
                    if prev is not None:
                        emit_pvhat_chunk(prev[0], prev[1], jc, prev[2])
                    if last:   # no next slot: run own PV-hat inline
                        emit_pvhat_chunk(w, g, jc, eTs)
                    if g == 1 and w >= 1:
                        emit_pj_piece(w - 1, jc)
                if prev is not None:
                    emit_finish(prev[0], prev[1])
                prev = (w, g, eTs)
            emit_finish(NW - 1, 1)
            for t4 in range(4):
                emit_pj_piece(NW - 1, t4)
    return nc


_CACHE = {}


def _get_nc():
    if "nc" not in _CACHE:
        nc = build_nc()
        nc.finalize()
        _CACHE["nc"] = nc
    return _CACHE["nc"]


def _host_lepe(v_win, conv_w, conv_b):
    """Depthwise 3x3 conv on [B, NW, C, 64, 8] window images (host, fp32).

    Each 64x8 window is zero-padded independently, matching the
    reference's per-window lax.conv on [B*nW, C, Hsp, Wsp]."""
    Bx, nw, C, H, W = v_win.shape
    pad = np.zeros((Bx, nw, C, H + 2, W + 2), np.float32)
    pad[:, :, :, 1:-1, 1:-1] = v_win
    out = np.broadcast_to(
        conv_b[None, None, :, None, None], v_win.shape).copy()
    cw = conv_w.reshape(C, 3, 3)
    for dy in range(3):
        for dx in range(3):
            out += cw[None, None, :, dy, dx, None, None] * \
                pad[:, :, :, dy:dy + H, dx:dx + W]
    return out


def _host_prep(qkv, scale, proj_w, proj_b, conv_w, conv_b):
    """Per-core input maps: all device layouts built host-side."""
    scale_v = float(np.asarray(scale).reshape(-1)[0])
    q_all = np.asarray(qkv[0], np.float32) * scale_v
    k_all = np.asarray(qkv[1], np.float32)
    v_all = np.asarray(qkv[2], np.float32)
    conv_w_h = np.asarray(conv_w, np.float32)
    conv_b_h = np.asarray(conv_b, np.float32)

    # weights (shared across cores). conv bias is folded into the lepe
    # plane itself (host conv adds it), so proj bias is just proj_b.
    # pw stationary: S[ch, g, k, c] = proj_w[k*128+c, g*128+ch]
    pw_h = np.ascontiguousarray(
        np.asarray(proj_w, np.float32).reshape(2, P, 2, P)
        .transpose(3, 2, 0, 1).reshape(P, 2 * DIM)).astype(ml_dtypes.bfloat16)
    pb_h = np.ascontiguousarray(
        np.asarray(proj_b, np.float32).reshape(2, P).T)

    # token reorder: l = h*64 + w*8 + s  ->  window w, t' = s*64 + h
    def to_win(x):
        xw = x.reshape(B, RESO, NW, STRIPE, DIM)          # [b, h, w, s, c]
        return np.ascontiguousarray(xw.transpose(0, 2, 3, 1, 4)).reshape(
            B, NW, WIN, DIM)                               # [b, w, s*64+h, c]

    qw = to_win(q_all)
    kw = to_win(k_all)
    vw = to_win(v_all)

    # lepe: per-window depthwise conv; vw is [b, w, (s h), c]
    v_win = vw.reshape(B, NW, STRIPE, RESO, DIM).transpose(0, 1, 4, 3, 2)
    lepe = _host_lepe(v_win, conv_w_h, conv_b_h)      # [b, w, c, h, s]
    lw = np.ascontiguousarray(lepe.transpose(0, 1, 4, 3, 2)).reshape(
        B, NW, WIN, DIM)                               # [b, w, (s h), c]

    # fused per-window blob [B, NW, P, WCOLS]: bf16 planes for qT/kT/lepeT,
    # fp16 bits for the vn-hat plane (PV runs in fp16 to match the
    # Schraudolph fp16 eT tiles).
    blob = np.zeros((B, NW, P, WCOLS), np.uint16)

    def bf16_bits(x):
        return x.astype(ml_dtypes.bfloat16).view(np.uint16)

    # qT/kT/lepeT: [p = ch within g, g*512 + t']
    for off, src in ((O_QT, qw), (O_KT, kw), (O_LP, lw)):
        t = src.transpose(0, 1, 3, 2).reshape(B, NW, 2, P, WIN)
        blob[:, :, :, off:off + 1024] = bf16_bits(
            t.transpose(0, 1, 3, 2, 4).reshape(B, NW, P, 1024))

    # vn-hat stationaries: [p = key within jc chunk, (g, jc, hp, 128)]
    # per slot s = hp%2: ones at cols [32s, 32s+32), v at [64+32s, +32).
    vn = np.zeros((B, NW, 2, 4, 4, P, P), np.float16)  # [b,w,g,jc,hp,key,col]
    vw4 = vw.reshape(B, NW, 4, P, NH, HD)              # [b,w,jc,key,head,ch]
    for g in range(2):
        for hp in range(4):
            s = hp % 2
            vn[:, :, g, :, hp, :, 32 * s:32 * s + 32] = 1.0
            vn[:, :, g, :, hp, :, 64 + 32 * s:96 + 32 * s] = \
                vw4[:, :, :, :, 4 * g + hp, :]
    blob[:, :, :, O_VN:O_VN + 4096] = vn.transpose(
        0, 1, 5, 2, 3, 4, 6).reshape(B, NW, P, 4096).view(np.uint16)
    blob_bf = blob.view(ml_dtypes.bfloat16)

    in_maps = []
    for b in range(B):
        in_maps.append({
            "win": np.ascontiguousarray(blob_bf[b]),
            "pw": pw_h, "pb": pb_h,
        })
    return in_maps


LAST_RESULTS = None


def kernel(qkv, scale, proj_w, proj_b, conv_w, conv_b):
    global LAST_RESULTS
    from concourse.bass_utils import run_bass_kernel_spmd
    nc = _get_nc()
    in_maps = _host_prep(qkv, scale, proj_w, proj_b, conv_w, conv_b)
    res = run_bass_kernel_spmd(nc, in_maps, core_ids=list(range(B)))
    LAST_RESULTS = res
    outs = []
    for b in range(B):
        o = np.asarray(res.results[b]["out"]).astype(np.float32)
        # device layout: [w, k, c, q] with t' = q = s*64 + h
        o = o.transpose(0, 3, 1, 2).reshape(NW, WIN, DIM)    # [w, t', c]
        o = o.reshape(NW, STRIPE, RESO, DIM)                 # [w, s, h, c]
        o = o.transpose(2, 0, 1, 3).reshape(L, DIM)          # [h*64+w*8+s, c]
        outs.append(o)
    return np.stack(outs, axis=0)
